# revision 11
# baseline (speedup 1.0000x reference)
"""Trainium2 Bass kernel for nn_Arm_82119774699744 (dense_cnn).

Reference: 501 overlapping width-500 crops of a [B=8, 36, 1001] signal, each
through 3x (conv15-valid -> BN -> ELU -> avgpool3) -> FC(4), accumulated over
crops, /501.

Algorithm (mathematically exact, validated vs reference in fp64):
  Convs are translation-equivariant, so every crop's conv output is a slice of
  one full-width conv. Only the avgpool phase (offset mod 3) differs, giving
  3 -> 9 -> 27 phase variants of the pooled streams. Crop s maps to phase
  m = s mod 27 and offset q = s // 27. This turns ~511 GFLOP into ~11 GFLOP.

  Host-side folds: BN into conv weights/bias; avgpool mean -> sum-of-3 with
  1/3 folded into next weights; ELU stored as elu+1 = relu(z) + min(exp(z), 1)
  with the -1 folded into the next layer's bias (rowsum of its weights); final
  FC + crop-sum + /501 folded into a masked reduction over the 27 phase
  streams of stage-3 pooled output.

Performance structure:
  - fp32r matmuls (full PE rate at moving-dim >= 256; even N required).
  - conv1 processes tap pairs via a partition-stacked copy of x (8 matmuls
    instead of 15 per chunk).
  - conv3's leftover input channels (128..143) are partition-stacked across
    taps via SBUF->SBUF DMA, turning 15 K=16 matmuls into 2 (K=128/112).
  - conv2/conv3 column chunks are aligned to pool phases so each chunk only
    depends on one phase-split pool op -> cross-stage pipelining on the PE.

Sharding: data-parallel over batch; core i handles batch element i. No
collectives; host scatters x and gathers the 8 [4]-vectors.
"""
import numpy as np

import concourse.bass as bass
import concourse.bacc as bacc
import concourse.mybir as mybir
import concourse.tile as tile
from concourse.bass_utils import run_bass_kernel_spmd

F32 = mybir.dt.float32
F32R = mybir.dt.float32r
AFT = mybir.ActivationFunctionType

EPS = 1e-5
B, C_IN, T, CROP, N_CROPS = 8, 36, 1001, 500, 501
N_CORES = 8

W1 = T - 14                       # 987 conv1 out cols
K1 = [(W1 - r) // 3 for r in range(3)]         # [329, 328, 328]
P1_PITCH = 329
P1_W = 3 * P1_PITCH               # 987
OFF1 = [0, 329, 658]

W2 = [K1[r] - 14 for r in range(3)]            # [315, 314, 314]
C2_CHUNKS = [(0, 316), (329, 314), (658, 314)]  # phase-aligned, even N
K2 = [(W2[i2 // 3] - (i2 % 3)) // 3 for i2 in range(9)]
P2_PITCH = 105
P2_W = 9 * P2_PITCH               # 945
C2_W = 976

W3 = [K2[i2] - 14 for i2 in range(9)]
C3_CHUNKS = [(0, 300), (315, 300), (630, 300)]  # phase-trio aligned, even N
K3 = [(W3[i3 // 3] - (i3 % 3)) // 3 for i3 in range(27)]
P3_PITCH = 30
P3_W = 27 * P3_PITCH              # 810
C3_W = 936
STACK_W = 932                     # conv3 stacked-rhs width (reads cols <= 929+2)

QPAD = 20
S_I3_CHUNKS = [(0, 14), (14, 13)]
S_W = 27 * QPAD                   # 540


def _m_of_i3(i3):
    r1, r2, r3 = (i3 // 9), (i3 // 3) % 3, i3 % 3
    return 9 * r3 + 3 * r2 + r1


for i3 in range(27):
    assert (N_CROPS - _m_of_i3(i3)) // 27 + 1 <= K3[i3] - 10


def _fv(tile_ap, rows, col0, dims):
    """Free-strided view: partition range + explicit [step,count] free dims."""
    base = tile_ap[rows[0]:rows[1], col0:col0 + 1]
    return bass.AP(base.tensor, base.offset, [list(base.ap[0])] + [list(d) for d in dims])


def build(mm_dtype=F32R):
    nc = bacc.Bacc(None, target_bir_lowering=False, debug=False)

    d_x = nc.dram_tensor("xb", [C_IN, T], mm_dtype, kind="ExternalInput")
    d_w1 = nc.dram_tensor("w1t", [72, 8 * 72], mm_dtype, kind="ExternalInput")
    d_b1 = nc.dram_tensor("b1t", [72, 1], F32, kind="ExternalInput")
    d_w2 = nc.dram_tensor("w2t", [72, 15 * 144], mm_dtype, kind="ExternalInput")
    d_b2 = nc.dram_tensor("b2t", [144, 1], F32, kind="ExternalInput")
    d_w3 = nc.dram_tensor("w3t", [128, 15 * 288], mm_dtype, kind="ExternalInput")
    d_w3sa = nc.dram_tensor("w3sat", [128, 288], mm_dtype, kind="ExternalInput")
    d_w3sb = nc.dram_tensor("w3sbt", [112, 288], mm_dtype, kind="ExternalInput")
    d_b3 = nc.dram_tensor("b3t", [288, 1], F32, kind="ExternalInput")
    d_wf = nc.dram_tensor("wft", [288, 11 * 4], mm_dtype, kind="ExternalInput")
    d_mask = nc.dram_tensor("maskt", [4, S_W], F32, kind="ExternalInput")
    d_out = nc.dram_tensor("outd", [4, 1], F32, kind="ExternalOutput")

    def mm(out, lhsT, rhs, start, stop):
        nc.tensor.matmul(out, lhsT, rhs, start=start, stop=stop)

    with tile.TileContext(nc) as tc:
        with (
            tc.tile_pool(name="const", bufs=1) as cpool,
            tc.tile_pool(name="acts", bufs=1) as apool,
            tc.tile_pool(name="scratch", bufs=4) as spool,
            tc.tile_pool(name="psum", bufs=5, space="PSUM") as ppool,
            tc.tile_pool(name="psum_s", bufs=2, space="PSUM") as pspool,
        ):
            # ---- input x, stacked by tap pairs: rows 36:72 = x shifted by 1 ----
            xs = cpool.tile([72, 1008], mm_dtype, tag="xs")
            nc.vector.memset(xs[:, 996:1008].bitcast(F32), 0.0)
            nc.sync.dma_start(xs[0:36, 0:T], d_x[:])
            nc.sync.dma_start(xs[36:72, 0:T - 1], d_x[:, 1:T])

            w1s = cpool.tile([72, 8 * 72], mm_dtype, tag="w1s")
            nc.sync.dma_start(w1s[:], d_w1[:])
            b1s = cpool.tile([72, 1], F32, tag="b1s")
            nc.sync.dma_start(b1s[:], d_b1[:])
            w2s = cpool.tile([72, 15 * 144], mm_dtype, tag="w2s")
            nc.sync.dma_start(w2s[:], d_w2[:])
            b2a = cpool.tile([128, 1], F32, tag="b2a")
            nc.sync.dma_start(b2a[:], d_b2[0:128, :])
            b2b = cpool.tile([16, 1], F32, tag="b2b")
            nc.sync.dma_start(b2b[:], d_b2[128:144, :])
            w3s = cpool.tile([128, 15 * 288], mm_dtype, tag="w3s")
            nc.sync.dma_start(w3s[:], d_w3[:])
            w3sa = cpool.tile([128, 288], mm_dtype, tag="w3sa")
            nc.sync.dma_start(w3sa[:], d_w3sa[:])
            w3sb = cpool.tile([112, 288], mm_dtype, tag="w3sb")
            nc.sync.dma_start(w3sb[:], d_w3sb[:])
            b3a = cpool.tile([128, 1], F32, tag="b3a")
            nc.sync.dma_start(b3a[:], d_b3[0:128, :])
            b3b = cpool.tile([128, 1], F32, tag="b3b")
            nc.sync.dma_start(b3b[:], d_b3[128:256, :])
            b3c = cpool.tile([32, 1], F32, tag="b3c")
            nc.sync.dma_start(b3c[:], d_b3[256:288, :])
            wfa = cpool.tile([128, 44], mm_dtype, tag="wfa")
            nc.sync.dma_start(wfa[:], d_wf[0:128, :])
            wfb = cpool.tile([128, 44], mm_dtype, tag="wfb")
            nc.sync.dma_start(wfb[:], d_wf[128:256, :])
            wfc = cpool.tile([32, 44], mm_dtype, tag="wfc")
            nc.sync.dma_start(wfc[:], d_wf[256:288, :])
            msk = cpool.tile([4, S_W], F32, tag="msk")
            nc.sync.dma_start(msk[:], d_mask[:])

            full1 = apool.tile([72, 992], F32, tag="full1")
            p1 = apool.tile([72, 988], mm_dtype, tag="p1")
            c2a = apool.tile([128, C2_W], F32, tag="c2a")
            c2b = apool.tile([16, C2_W], F32, tag="c2b")
            p2a = apool.tile([128, 948], mm_dtype, tag="p2a")
            p2b = apool.tile([16, 948], mm_dtype, tag="p2b")
            stka = apool.tile([128, STACK_W], mm_dtype, tag="stka")
            stkb = apool.tile([112, STACK_W], mm_dtype, tag="stkb")
            c3a = apool.tile([128, C3_W], F32, tag="c3a")
            c3b = apool.tile([128, C3_W], F32, tag="c3b")
            c3c = apool.tile([32, C3_W], F32, tag="c3c")
            p3a = apool.tile([128, P3_W], mm_dtype, tag="p3a")
            p3b = apool.tile([128, P3_W], mm_dtype, tag="p3b")
            p3c = apool.tile([32, P3_W], mm_dtype, tag="p3c")
            sm = apool.tile([4, S_W], F32, tag="sm")
            red = apool.tile([4, 1], F32, tag="red")

            def elu1(ps_ap, rows, dst, dcol0, L, bias):
                """dst[:, dcol0:dcol0+L] = elu(ps + bias) + 1."""
                et = spool.tile([128, 512], F32, tag="et")
                d = dst[0:rows, dcol0:dcol0 + L]
                nc.scalar.activation(d, ps_ap, AFT.Relu, bias=bias[0:rows, 0:1])
                nc.scalar.activation(et[0:rows, 0:L], ps_ap, AFT.Exp,
                                     bias=bias[0:rows, 0:1])
                nc.vector.tensor_scalar_min(et[0:rows, 0:L], et[0:rows, 0:L], 1.0)
                nc.vector.tensor_add(d, d, et[0:rows, 0:L])

            # ================= stage 1: conv1 [36 -> 72], tap pairs =============
            for (n0, nl) in ((0, 494), (494, 494)):
                ps = ppool.tile([72, 494], F32, tag="ps")
                for j in range(8):
                    mm(ps[:, 0:nl], w1s[:, j * 72:(j + 1) * 72],
                       xs[:, n0 + 2 * j:n0 + 2 * j + nl],
                       start=(j == 0), stop=(j == 7))
                elu1(ps[:, 0:nl], 72, full1, n0, nl, b1s)
            nc.vector.memset(full1[:, W1 + 1:992], 0.0)

            # pool1, phase-split
            for r in range(3):
                a0 = _fv(full1[:], (0, 72), r + 0, [[3, P1_PITCH]])
                a1 = _fv(full1[:], (0, 72), r + 1, [[3, P1_PITCH]])
                a2 = _fv(full1[:], (0, 72), r + 2, [[3, P1_PITCH]])
                o = p1[:, OFF1[r]:OFF1[r] + P1_PITCH]
                nc.vector.tensor_add(o, a0, a1)
                nc.vector.tensor_add(o, o, a2)
            nc.vector.memset(p1[:, 987:988].bitcast(F32), 0.0)

            # ================= stage 2: conv2 [72 -> 144], phase chunks =========
            for (n0, nl) in C2_CHUNKS:
                for (m0, ml, dst, bias) in ((0, 128, c2a, b2a), (128, 16, c2b, b2b)):
                    ps = ppool.tile([128, 316], F32, tag="ps")
                    for k in range(15):
                        mm(ps[0:ml, 0:nl], w2s[:, k * 144 + m0:k * 144 + m0 + ml],
                           p1[:, n0 + k:n0 + k + nl], start=(k == 0), stop=(k == 14))
                    elu1(ps[0:ml, 0:nl], ml, dst, n0, nl, bias)
            for t_ in (c2a, c2b):
                nc.vector.memset(t_[:, 316:329], 0.0)
                nc.vector.memset(t_[:, 643:658], 0.0)
                nc.vector.memset(t_[:, 972:C2_W], 0.0)

            # pool2, split per r1
            for r1 in range(3):
                for (src, dst, rows) in ((c2a, p2a, 128), (c2b, p2b, 16)):
                    a0 = _fv(src[:], (0, rows), OFF1[r1] + 0, [[1, 3], [3, P2_PITCH]])
                    a1 = _fv(src[:], (0, rows), OFF1[r1] + 1, [[1, 3], [3, P2_PITCH]])
                    a2 = _fv(src[:], (0, rows), OFF1[r1] + 2, [[1, 3], [3, P2_PITCH]])
                    o = _fv(dst[:], (0, rows), 315 * r1, [[P2_PITCH, 3], [1, P2_PITCH]])
                    nc.vector.tensor_add(o, a0, a1)
                    nc.vector.tensor_add(o, o, a2)
            nc.vector.memset(p2a[:, P2_W:948].bitcast(F32), 0.0)
            nc.vector.memset(p2b[:, P2_W:948].bitcast(F32), 0.0)

            # conv3 stacked leftover channels: stka rows 16j+c = p2b[c, .+j]
            for j in range(8):
                nc.sync.dma_start(stka[16 * j:16 * j + 16, :], p2b[0:16, j:j + STACK_W])
            for j in range(8, 15):
                jj = j - 8
                nc.sync.dma_start(stkb[16 * jj:16 * jj + 16, :], p2b[0:16, j:j + STACK_W])

            # ================= stage 3: conv3 [144 -> 288] ======================
            for (n0, nl) in C3_CHUNKS:
                for (m0, ml, dst, bias) in ((0, 128, c3a, b3a), (128, 128, c3b, b3b),
                                            (256, 32, c3c, b3c)):
                    ps = ppool.tile([128, 300], F32, tag="ps")
                    for k in range(15):
                        mm(ps[0:ml, 0:nl], w3s[:, k * 288 + m0:k * 288 + m0 + ml],
                           p2a[:, n0 + k:n0 + k + nl], start=(k == 0), stop=False)
                    mm(ps[0:ml, 0:nl], w3sa[:, m0:m0 + ml], stka[:, n0:n0 + nl],
                       start=False, stop=False)
                    mm(ps[0:ml, 0:nl], w3sb[:, m0:m0 + ml], stkb[:, n0:n0 + nl],
                       start=False, stop=True)
                    elu1(ps[0:ml, 0:nl], ml, dst, n0, nl, bias)
            for t_ in (c3a, c3b, c3c):
                nc.vector.memset(t_[:, 300:315], 0.0)
                nc.vector.memset(t_[:, 615:630], 0.0)
                nc.vector.memset(t_[:, 930:C3_W], 0.0)

            # pool3, split per phase trio g (i2 = 3g..3g+2)
            for g in range(3):
                for (src, dst, rows) in ((c3a, p3a, 128), (c3b, p3b, 128), (c3c, p3c, 32)):
                    gi = [[P2_PITCH, 3], [1, 3], [3, P3_PITCH]]
                    go = [[3 * P3_PITCH, 3], [P3_PITCH, 3], [1, P3_PITCH]]
                    a0 = _fv(src[:], (0, rows), 315 * g + 0, gi)
                    a1 = _fv(src[:], (0, rows), 315 * g + 1, gi)
                    a2 = _fv(src[:], (0, rows), 315 * g + 2, gi)
                    o = _fv(dst[:], (0, rows), 270 * g, go)
                    nc.vector.tensor_add(o, a0, a1)
                    nc.vector.tensor_add(o, o, a2)

            # ============ S stage: conv11 with wfc over 27 phase streams ========
            for ci, (i30, ni3) in enumerate(S_I3_CHUNKS):
                ps = pspool.tile([4, 16 * QPAD], F32, tag="pss")
                out_ap = ps[:, 0:ni3 * QPAD].rearrange("p (a b) -> p a b", b=QPAD)
                i = 0
                nmm = 33
                for j in range(11):
                    for (wt, src, rl) in ((wfa, p3a, 128), (wfb, p3b, 128), (wfc, p3c, 32)):
                        rhs = _fv(src[:], (0, rl), 30 * i30 + j, [[30, ni3], [1, QPAD]])
                        mm(out_ap, wt[:, j * 4:(j + 1) * 4], rhs,
                           start=(i == 0), stop=(i == nmm - 1))
                        i += 1
                c0 = i30 * QPAD
                nc.vector.tensor_mul(sm[:, c0:c0 + ni3 * QPAD], ps[:, 0:ni3 * QPAD],
                                     msk[:, c0:c0 + ni3 * QPAD])

            nc.vector.reduce_sum(red[:], sm[:], axis=mybir.AxisListType.X)
            nc.sync.dma_start(d_out[:], red[:])

    nc.compile()
    return nc


# ----------------------- host side -----------------------

def _fold_bn(w, b, g, be, m, v):
    s = g.astype(np.float64) / np.sqrt(v.astype(np.float64) + EPS)
    return w.astype(np.float64) * s[:, None, None], \
        (b.astype(np.float64) - m.astype(np.float64)) * s + be.astype(np.float64)


def prep_inputs(inputs):
    w1, b1 = _fold_bn(inputs['w1'][:, :, 0, :], inputs['b1'], inputs['g1'],
                      inputs['be1'], inputs['m1'], inputs['v1'])
    w2, b2 = _fold_bn(inputs['w2'][:, :, 0, :], inputs['b2'], inputs['g2'],
                      inputs['be2'], inputs['m2'], inputs['v2'])
    w3, b3 = _fold_bn(inputs['w3'][:, :, 0, :], inputs['b3'], inputs['g3'],
                      inputs['be3'], inputs['m3'], inputs['v3'])
    wfc = inputs['wfc'].astype(np.float64)
    bfc = inputs['bfc'].astype(np.float64)

    w2s = w2 / 3.0
    b2s = b2 - w2.sum((1, 2))
    w3f = w3 / 3.0
    b3s = b3 - w3.sum((1, 2))
    wfc3 = wfc.reshape(4, 288, 11) / 3.0
    Ko = bfc - wfc.reshape(4, 288, 11).sum((1, 2))

    f32 = lambda a: np.ascontiguousarray(a, np.float32)

    # conv1 tap-pair weights: block j rows 0:36 = tap 2j, rows 36:72 = tap 2j+1
    w1p = np.zeros((72, 8 * 72), np.float64)
    for j in range(8):
        w1p[0:36, j * 72:(j + 1) * 72] = w1[:, :, 2 * j].T
        if 2 * j + 1 < 15:
            w1p[36:72, j * 72:(j + 1) * 72] = w1[:, :, 2 * j + 1].T

    # conv3 stacked leftover weights: w3sa rows 16j+c = w3f[co, 128+c, j]
    w3sa = np.zeros((128, 288), np.float64)
    for j in range(8):
        w3sa[16 * j:16 * j + 16, :] = w3f[:, 128:144, j].T
    w3sb = np.zeros((112, 288), np.float64)
    for j in range(8, 15):
        w3sb[16 * (j - 8):16 * (j - 8) + 16, :] = w3f[:, 128:144, j].T

    shifts = {0, *range(2, 502)}
    mask = np.zeros((4, S_W), np.float64)
    for i3 in range(27):
        m = _m_of_i3(i3)
        for q in range(QPAD):
            s = 27 * q + m
            if s in shifts and q <= (N_CROPS - m) // 27:
                mask[:, i3 * QPAD + q] = 1.0 / N_CROPS

    common = {
        "w1t": f32(w1p),
        "b1t": f32(b1.reshape(72, 1)),
        "w2t": f32(w2s.transpose(1, 2, 0).reshape(72, 15 * 144)),
        "b2t": f32(b2s.reshape(144, 1)),
        "w3t": f32(w3f[:, 0:128, :].transpose(1, 2, 0).reshape(128, 15 * 288)),
        "w3sat": f32(w3sa),
        "w3sbt": f32(w3sb),
        "b3t": f32(b3s.reshape(288, 1)),
        "wft": f32(wfc3.transpose(1, 2, 0).reshape(288, 44)),
        "maskt": f32(mask),
    }
    x = np.asarray(inputs['x'], np.float32)
    in_maps = []
    for c in range(N_CORES):
        m = dict(common)
        m["xb"] = np.ascontiguousarray(x[c, :, 0, :])
        in_maps.append(m)
    return in_maps, f32(Ko)


_NC_CACHE = {}


def run(inputs, mm_dtype=F32R, **kw):
    key = str(mm_dtype)
    if key not in _NC_CACHE:
        _NC_CACHE[key] = build(mm_dtype)
    nc = _NC_CACHE[key]
    in_maps, Ko = prep_inputs(inputs)
    res = run_bass_kernel_spmd(nc, in_maps, core_ids=list(range(N_CORES)), **kw)
    out = np.stack([r["outd"].reshape(4) for r in res.results]) + Ko[None, :]
    return out.astype(np.float32), res


def kernel(**inputs):
    out, _ = run(inputs)
    return out
